# revision 11
# baseline (speedup 1.0000x reference)
"""DLRM-style EBM forward (embedding gather + 2-layer MLP) on 8 TRN2 cores.

Batch-sharded data parallel: each core handles 2048 of 16384 rows and holds a
full copy of the embedding tables in its DRAM space. The gather is done with
large indirect DMAs (SWDGE), the MLP runs in fp32r on the PE at full rate.

Self-contained: only imports the system-installed concourse stack.
"""

import sys

sys.path.insert(0, "/opt/trn_rl_repo")

from contextlib import ExitStack

import numpy as np

import concourse.bass as bass
import concourse.tile as tile
from concourse import bacc, mybir
from concourse.bass_utils import run_bass_kernel_spmd
from concourse.masks import make_identity

# Problem shapes (hardcoded per contract)
B = 16384
NUM = 13
C = 26
V = 100000
D = 16
H = 1024
FEAT = NUM + C * D  # 429

NCORES = 8
BC = B // NCORES  # 2048 batch rows per core
P = 128
NT = BC // P  # 16 batch tiles of 128 per core
NG = 4  # gather chunks per core
TPG = NT // NG  # 4 batch tiles per gather chunk
CATF = C * D  # 416 categorical features
K3 = FEAT - 3 * P  # 45 rows in the last K chunk (32 cat + 13 num)
CAT3 = CATF - 3 * P  # 32 cat features in the last chunk
HC = H // P  # 8 hidden chunks of 128
NBC = NT // TPG  # 4 batch column blocks of 512
BW = BC // NBC  # 512

f32 = mybir.dt.float32
f32r = mybir.dt.float32r
bf16 = mybir.dt.bfloat16
i32 = mybir.dt.int32

_CACHE = {}


def _build(taps=False):
    """Build + schedule the single-core SPMD Bass program."""
    nc = bacc.Bacc("TRN2", target_bir_lowering=False, debug=False)

    emb = nc.dram_tensor("emb", [C * V, D], f32, kind="ExternalInput")
    idx = nc.dram_tensor("idx", [P, NT * C], i32, kind="ExternalInput")
    xnt = nc.dram_tensor("xnt", [NUM, BC], f32, kind="ExternalInput")
    w1 = nc.dram_tensor("w1", [FEAT, H], f32, kind="ExternalInput")
    w2 = nc.dram_tensor("w2", [P, HC], f32, kind="ExternalInput")
    b1 = nc.dram_tensor("b1", [P, HC], f32, kind="ExternalInput")
    b2 = nc.dram_tensor("b2", [1, 1], f32, kind="ExternalInput")
    out = nc.dram_tensor("out", [1, BC], f32, kind="ExternalOutput")

    with tile.TileContext(nc) as tc, ExitStack() as ctx:
        const = ctx.enter_context(tc.tile_pool(name="const", bufs=1))
        gpool = ctx.enter_context(tc.tile_pool(name="gath", bufs=1))
        hpool = ctx.enter_context(tc.tile_pool(name="ht", bufs=1))
        h2pool = ctx.enter_context(tc.tile_pool(name="h2", bufs=3))
        ps1 = ctx.enter_context(tc.tile_pool(name="ps1", bufs=4, space="PSUM"))
        pstr = ctx.enter_context(tc.tile_pool(name="pstr", bufs=2, space="PSUM"))
        ps2 = ctx.enter_context(tc.tile_pool(name="ps2", bufs=2, space="PSUM"))

        # --- constants / weights ---
        ident = const.tile([P, P], f32, tag="ident")
        make_identity(nc, ident[:])

        w1t = []
        for k in range(4):
            t = const.tile([P, H], bf16, tag=f"w1_{k}", name=f"w1t{k}")
            rows = P if k < 3 else K3
            nc.gpsimd.dma_start(t[:rows, :], w1[k * P : k * P + rows, :])
            w1t.append(t)
        w2t = const.tile([P, HC], bf16, tag="w2")
        nc.gpsimd.dma_start(w2t[:], w2[:, :])
        b1t = const.tile([P, HC], f32, tag="b1")
        nc.sync.dma_start(b1t[:], b1[:, :])
        b2t = const.tile([1, 1], f32, tag="b2")
        nc.sync.dma_start(b2t[:], b2[:, :])

        idxt = const.tile([P, NT * C], i32, tag="idx")
        nc.sync.dma_start(idxt[:], idx[:, :])

        # hT: features on partitions, batch on free dim.
        # rows 0..415 = cat features (transposed gather), hT3 rows 32..44 = x_num.
        hT = [hpool.tile([P, BC], bf16, tag=f"hT{k}", name=f"hT{k}") for k in range(4)]
        nc.gpsimd.dma_start(hT[3][CAT3 : CAT3 + NUM, :], xnt[:, :])

        # --- gather: per (batch tile, feature) indirect DMA, 128 rows each.
        # HW semantics: one dynamic offset per partition per instruction.
        gts = []
        for g in range(NG):
            gt = gpool.tile([P, TPG * CATF], f32, tag=f"g{g}", name=f"gath{g}")
            for q in range(TPG):
                t = g * TPG + q
                for c in range(C):
                    nc.gpsimd.indirect_dma_start(
                        out=gt[:, q * CATF + c * D : q * CATF + (c + 1) * D],
                        out_offset=None,
                        in_=emb[:, :],
                        in_offset=bass.IndirectOffsetOnAxis(
                            ap=idxt[:, t * C + c : t * C + c + 1], axis=0
                        ),
                    )
            gts.append(gt)

        osb = const.tile([1, BC], f32, tag="osb")

        for g in range(NG):
            gt = gts[g]
            # transpose chunk g into hT[0..3] columns [g*512, (g+1)*512)
            for c in range(4):
                cw = P if c < 3 else CAT3
                ptr = pstr.tile([P, BW], f32, tag="ptr")
                for q in range(TPG):
                    nc.tensor.transpose(
                        out=ptr[:cw, q * P : (q + 1) * P],
                        in_=gt[:, q * CATF + c * P : q * CATF + c * P + cw],
                        identity=ident[:],
                    )
                nc.vector.tensor_copy(
                    out=hT[c][:cw, g * BW : (g + 1) * BW], in_=ptr[:cw, :]
                )

            # MLP on batch block g (columns g*512..)
            bs = slice(g * BW, (g + 1) * BW)
            p2 = ps2.tile([1, BW], f32, tag="p2")
            for hc in range(HC):
                p1 = ps1.tile([P, BW], f32, tag="p1")
                for k in range(4):
                    rows = P if k < 3 else K3
                    nc.tensor.matmul(
                        out=p1[:],
                        lhsT=w1t[k][:rows, hc * P : (hc + 1) * P],
                        rhs=hT[k][:rows, bs],
                        start=(k == 0),
                        stop=(k == 3),
                    )
                h2 = h2pool.tile([P, BW], bf16, tag="h2")
                nc.scalar.activation(
                    h2[:],
                    p1[:],
                    mybir.ActivationFunctionType.Relu,
                    bias=b1t[:, hc : hc + 1],
                )
                nc.tensor.matmul(
                    out=p2[:],
                    lhsT=w2t[:, hc : hc + 1],
                    rhs=h2[:],
                    start=(hc == 0),
                    stop=(hc == HC - 1),
                )
            nc.scalar.activation(
                osb[:, bs],
                p2[:],
                mybir.ActivationFunctionType.Identity,
                bias=b2t[:, 0:1],
            )

        nc.sync.dma_start(out[:, :], osb[:])

        if taps:
            dbg_g = nc.dram_tensor(
                "dbg_g", [P, TPG * CATF], f32, kind="ExternalOutput"
            )
            nc.sync.dma_start(dbg_g[:, :], gts[0][:])
            dbg_h = nc.dram_tensor("dbg_h", [P, BC], f32, kind="ExternalOutput")
            nc.gpsimd.dma_start(dbg_h[:, :], hT[0][:])

    nc.compile()
    return nc


def get_program():
    if "nc" not in _CACHE:
        _CACHE["nc"] = _build()
    return _CACHE["nc"]


def prep_inputs(x_num, x_cat, emb_w, emb_b, W1, b1, W2, b2):
    """Host-side shard + layout prep. Returns in_maps for the 8 cores."""
    x_num = np.asarray(x_num, dtype=np.float32)
    x_cat = np.asarray(x_cat).astype(np.int64)
    emb_w = np.asarray(emb_w, dtype=np.float32)
    emb_b = np.asarray(emb_b, dtype=np.float32)
    W1 = np.asarray(W1, dtype=np.float32)
    b1 = np.asarray(b1, dtype=np.float32)
    W2 = np.asarray(W2, dtype=np.float32)
    b2 = np.asarray(b2, dtype=np.float32)

    # combined row indices into the flat [C*V, D] table
    comb = x_cat + (np.arange(C, dtype=np.int64) * V)[None, :]  # [B, C]
    comb = comb.astype(np.int32)

    # feature permutation [cat | num]; fold emb_b into b1
    W1p = np.concatenate([W1[NUM:], W1[:NUM]], axis=0)  # [429, H]
    b1_eff = (
        b1.astype(np.float64)
        + emb_b.reshape(-1).astype(np.float64) @ W1[NUM:].astype(np.float64)
    ).astype(np.float32)

    emb_flat = np.ascontiguousarray(emb_w.reshape(C * V, D))
    w2_mat = np.ascontiguousarray(W2.reshape(HC, P).T)
    b1_mat = np.ascontiguousarray(b1_eff.reshape(HC, P).T)
    b2_mat = b2.reshape(1, 1)

    in_maps = []
    for i in range(NCORES):
        rows = slice(i * BC, (i + 1) * BC)
        idx_core = (
            comb[rows].reshape(NT, P, C).transpose(1, 0, 2).reshape(P, NT * C)
        )
        in_maps.append(
            {
                "emb": emb_flat,
                "idx": np.ascontiguousarray(idx_core),
                "xnt": np.ascontiguousarray(x_num[rows].T),
                "w1": W1p,
                "w2": w2_mat,
                "b1": b1_mat,
                "b2": b2_mat,
            }
        )
    return in_maps


def _install_ntff_shim():
    """Provide antenv.axon_hooks (missing in this image) so trace=True works."""
    import types

    try:
        from antenv.axon_hooks import get_axon_ntff_profile_hook  # noqa: F401

        return
    except ImportError:
        pass
    import antenv

    sys.path.insert(0, "/root/.axon_site")
    from trn_agent_boot.trn_boot import _ntff_profile_via_ctypes

    mod = types.ModuleType("antenv.axon_hooks")
    state = {"hook": _ntff_profile_via_ctypes("/opt/axon/libaxon_pjrt.so")}
    mod.get_axon_ntff_profile_hook = lambda: state["hook"]
    mod.set_axon_ntff_profile_hook = lambda h: state.update(hook=h)
    sys.modules["antenv.axon_hooks"] = mod
    antenv.axon_hooks = mod


def kernel(x_num, x_cat, emb_w, emb_b, W1, b1, W2, b2, _trace=False):
    if _trace:
        _install_ntff_shim()
    nc = get_program()
    in_maps = prep_inputs(x_num, x_cat, emb_w, emb_b, W1, b1, W2, b2)
    res = run_bass_kernel_spmd(nc, in_maps, list(range(NCORES)), trace=_trace)
    outp = np.concatenate([res.results[i]["out"].reshape(-1) for i in range(NCORES)])
    if _trace:
        kernel.last_exec_time_ns = res.exec_time_ns
        kernel.last_results = res
    return outp
